# revision 1
# baseline (speedup 1.0000x reference)
"""Trainium2 Bass kernel for nn_CustomDistanceLayer (variance-weighted distance
+ 32x32 stride-1 box-sum pooling).

Reference computation (shapes hardcoded):
    kernel = tile(input_image[32,32] -> [4096,4096])
    dist   = (kernel - som_matrix)^2 / (som_running_variances + 1e-8)
    out    = 32x32 valid box-sum of dist -> [4065, 4065]

Strategy (8 NeuronCores, SPMD, row-sharded with 31-row halo):
  * Every core runs the SAME program on a 543-row slab (512 output rows + 31
    halo rows); slab starts overlap slightly so all shapes are uniform.
  * Host passes a per-core pre-negated tiled-image block (handles slab start
    not being a multiple of 32 via a row roll).
  * Per core: diff = som + (-kern) on DVE, Square on ScalarE, 1/var via the
    DVE fast reciprocal, horizontal sliding box-sum in a single
    tensor_tensor_scan pass (h[j] = h[j-1] + d[j+31] - d[j-1], fp32 state,
    bf16 stream), vertical 32-row band-sum as two accumulating bf16 matmuls
    against banded 0/1 weights on the TensorEngine, ScalarE PSUM drain,
    DMA out split over the three DMA trigger paths (som on the SP HWDGE
    ring, var on SWDGE, outputs on the ACT HWDGE ring).  (A DMA-accumulate variant that adds som onto a
    prefilled -kern tile in the SDMA CCE exists behind use_accum=True, but it
    crashes the device under the axon/bass2jax path, so it stays off.)
"""
import numpy as np
import ml_dtypes

import concourse.bass as bass
import concourse.mybir as mybir
import concourse.tile as tile
from concourse import bacc
from concourse.bass_utils import run_bass_kernel_spmd

K = 32
HH = 4096
OUT = HH - K + 1  # 4065
N_CORES = 8
OUT_ROWS = 512
DIST_ROWS = OUT_ROWS + K - 1  # 543
STARTS = [round(c * (OUT - OUT_ROWS) / (N_CORES - 1)) for c in range(N_CORES)]

PB = [128, 128, 128, 128, DIST_ROWS - 512]  # partition rows per block
RB = [0, 128, 256, 384, 512]
N_BLK = len(PB)
N_OB = 4  # output row-blocks of 128

F32 = mybir.dt.float32
BF16 = mybir.dt.bfloat16

# column chunks for the vertical matmul (PSUM free-dim limit 512 for f32 out)
JCHUNKS = [(j, min(512, OUT - j)) for j in range(0, OUT, 512)]

_PROGRAM_CACHE = {}


def _band_w1():
    k = np.arange(128)[:, None]
    m = np.arange(128)[None, :]
    return ((m <= k) & (k <= m + K - 1)).astype(ml_dtypes.bfloat16)


def _band_w2():
    kk = np.arange(K - 1)[:, None]
    m = np.arange(128)[None, :]
    return (m >= kk + 128 - (K - 1)).astype(ml_dtypes.bfloat16)


def build_program(use_accum=False, repeat=1, gps_ops=0, interleave=True):
    """gps_ops: how many of the two tensor-tensor passes run on GPSIMD
    (0: none, 1: the kern-add, 2: add + the sq*w multiply)."""
    nc = bacc.Bacc("TRN2", target_bir_lowering=False, debug=False)
    som = nc.dram_tensor("som", [DIST_ROWS, HH], F32, kind="ExternalInput").ap()
    var = nc.dram_tensor("var", [DIST_ROWS, HH], F32, kind="ExternalInput").ap()
    nkern = nc.dram_tensor("nkern", [128, HH], F32, kind="ExternalInput").ap()
    w1d = nc.dram_tensor("w1", [128, 128], BF16, kind="ExternalInput").ap()
    w2d = nc.dram_tensor("w2", [K - 1, 128], BF16, kind="ExternalInput").ap()
    out = nc.dram_tensor("out", [OUT_ROWS, OUT], F32, kind="ExternalOutput").ap()

    with tile.TileContext(nc) as tc:
        with (
            tc.tile_pool(name="const", bufs=1) as constp,
            tc.tile_pool(name="som", bufs=3) as somp,
            tc.tile_pool(name="var", bufs=2) as varp,
            tc.tile_pool(name="w", bufs=1) as wp,
            tc.tile_pool(name="d", bufs=2) as dp,
            tc.tile_pool(name="h0", bufs=2) as h0p,
            tc.tile_pool(name="h", bufs=3) as hp,
            tc.tile_pool(name="outp", bufs=2) as outp,
            tc.tile_pool(name="psum", bufs=8, space="PSUM") as psump,
        ):
            nkern_sb = constp.tile([128, HH], F32)
            nc.sync.dma_start(nkern_sb[:], nkern[:, :])
            w1_sb = constp.tile([128, 128], BF16)
            nc.sync.dma_start(w1_sb[:], w1d[:, :])
            w2_sb = constp.tile([K - 1, 128], BF16)
            nc.sync.dma_start(w2_sb[:], w2d[:, :])

            add_eng = nc.gpsimd if gps_ops >= 1 else nc.vector
            mul_eng = nc.gpsimd if gps_ops >= 2 else nc.vector

            for _ in range(repeat):
                h_blocks = []

                def emit_block(b):
                    p = PB[b]
                    rows = slice(RB[b], RB[b] + p)
                    som_t = somp.tile([p, HH], F32)
                    # som on the SP HWDGE ring, var on the ACT HWDGE ring,
                    # outputs on SWDGE: three DMA paths run in parallel
                    nc.sync.dma_start(som_t[:], som[rows, :])
                    # diff = som + (-kern), in place over som
                    add_eng.tensor_add(som_t[:], som_t[:], nkern_sb[:p, :])
                    var_t = varp.tile([p, HH], F32)
                    nc.gpsimd.dma_start(var_t[:], var[rows, :])
                    w_t = wp.tile([p, HH], F32)
                    nc.vector.reciprocal_approx_fast(w_t[:], var_t[:])
                    # sq = diff^2 in place (ScalarE)
                    nc.scalar.activation(
                        som_t[:], som_t[:], mybir.ActivationFunctionType.Square
                    )
                    # d = sq * w -> bf16 (16-bit scan input; scan state is fp32)
                    d_t = dp.tile([p, HH], BF16)
                    mul_eng.tensor_mul(d_t[:], som_t[:], w_t[:])
                    # sliding 32-wide window sum in ONE scan pass:
                    #   h[0] = sum(d[0:32]);  h[j] = h[j-1] + d[j+31] - d[j-1]
                    # (bf16 d errors cancel exactly when an element leaves the
                    # window; only fp32 state rounding accumulates)
                    h_t = hp.tile([p, OUT], BF16)
                    h0 = h0p.tile([p, 1], F32)
                    nc.vector.tensor_reduce(
                        h0[:], d_t[:, 0:K], mybir.AxisListType.X, mybir.AluOpType.add
                    )
                    nc.vector.tensor_copy(h_t[:, 0:1], h0[:])
                    nc.vector.tensor_tensor_scan(
                        h_t[:, 1:OUT],
                        d_t[:, K:HH],
                        d_t[:, 0 : OUT - 1],
                        initial=h0[:],
                        op0=mybir.AluOpType.add,
                        op1=mybir.AluOpType.subtract,
                    )
                    h_blocks.append(h_t)

                def emit_mm_group(ib):
                    out_t = outp.tile([128, OUT], F32)
                    psums = []
                    for j0, jw in JCHUNKS:
                        ps = psump.tile([128, jw], F32)
                        nc.tensor.matmul(
                            ps[:],
                            w1_sb[:],
                            h_blocks[ib][:, j0 : j0 + jw],
                            start=True,
                            stop=False,
                        )
                        psums.append(ps)
                    for (j0, jw), ps in zip(JCHUNKS, psums):
                        nc.tensor.matmul(
                            ps[:],
                            w2_sb[:],
                            h_blocks[ib + 1][: K - 1, j0 : j0 + jw],
                            start=False,
                            stop=True,
                        )
                    for (j0, jw), ps in zip(JCHUNKS, psums):
                        nc.scalar.copy(out_t[:, j0 : j0 + jw], ps[:])
                    nc.scalar.dma_start(out[ib * 128 : (ib + 1) * 128, :], out_t[:])

                if interleave:
                    # b0, b1, mm0, b2, mm1, b3, mm2, b4, mm3
                    emit_block(0)
                    emit_block(1)
                    for ib in range(N_OB):
                        if ib + 2 < N_BLK:
                            emit_block(ib + 2)
                        emit_mm_group(ib)
                else:
                    for b in range(N_BLK):
                        emit_block(b)
                    for ib in range(N_OB):
                        emit_mm_group(ib)

    nc.compile()
    return nc


def get_program(use_accum=False, repeat=1, gps_ops=0, interleave=True):
    key = (use_accum, repeat, gps_ops, interleave)
    if key not in _PROGRAM_CACHE:
        _PROGRAM_CACHE[key] = build_program(use_accum, repeat, gps_ops, interleave)
    return _PROGRAM_CACHE[key]


def make_in_maps(input_image, som_matrix, som_running_variances):
    img = np.ascontiguousarray(np.asarray(input_image, dtype=np.float32))
    som = np.ascontiguousarray(np.asarray(som_matrix, dtype=np.float32))
    var = np.ascontiguousarray(np.asarray(som_running_variances, dtype=np.float32))
    w1 = np.ascontiguousarray(_band_w1())
    w2 = np.ascontiguousarray(_band_w2())
    in_maps = []
    for c in range(N_CORES):
        s = STARTS[c]
        # slab-local row i is global row s+i -> kern row img[(s+i) % K]
        negkern = np.ascontiguousarray(
            (-np.tile(np.roll(img, -(s % K), axis=0), (128 // K, HH // K))).astype(
                np.float32
            )
        )
        in_maps.append(
            {
                "som": np.ascontiguousarray(som[s : s + DIST_ROWS]),
                "var": np.ascontiguousarray(var[s : s + DIST_ROWS]),
                "nkern": negkern,
                "w1": w1,
                "w2": w2,
            }
        )
    return in_maps


def assemble(results):
    out_full = np.empty((OUT, OUT), np.float32)
    for c in range(N_CORES):
        lo = STARTS[c]
        hi = STARTS[c + 1] if c < N_CORES - 1 else OUT
        out_full[lo:hi] = results[c]["out"][: hi - lo]
    return out_full


def kernel(input_image, som_matrix, som_running_variances):
    nc = get_program()
    in_maps = make_in_maps(input_image, som_matrix, som_running_variances)
    res = run_bass_kernel_spmd(nc, in_maps, core_ids=list(range(N_CORES)))
    return assemble(res.results)



# revision 18
# speedup vs baseline: 3.3324x; 3.3324x over previous
"""Trainium2 Bass kernel for nn_CustomDistanceLayer (variance-weighted distance
+ 32x32 stride-1 box-sum pooling).

Reference computation (shapes hardcoded):
    kernel = tile(input_image[32,32] -> [4096,4096])
    dist   = (kernel - som_matrix)^2 / (som_running_variances + 1e-8)
    out    = 32x32 valid box-sum of dist -> [4065, 4065]

Strategy (8 NeuronCores, SPMD, row-sharded with 31-row halo):
  * Every core runs the SAME program on a 543-row slab (512 output rows + 31
    halo rows); slab starts overlap slightly so all shapes are uniform.
  * fp16 everywhere on the wire: host ships som and w = 1/(var+eps) as fp16
    (tolerance is 2e-2; fp16 end-to-end sims at ~6e-4), output returns fp16
    and is upcast on host.  HBM traffic per core: 4.5 MB som + 4.5 MB w in,
    4.2 MB out.
  * Per core, per 128-row block: diff = som + (-kern) (Pool or DVE), sq =
    diff^2 (ScalarE), d = sq*w (DVE, fp16 2x mode), horizontal sliding
    box-sum in one tensor_tensor_scan pass (h[j] = h[j-1] + d[j+31] - d[j-1],
    fp32 state), vertical 32-row band-sum as two accumulating matmuls against
    banded 0/1 weights (TensorE), ScalarE PSUM drain -> fp16, DMA out.
  * The 31 halo rows are folded into a [124, 1055] tile (4 column segments of
    31 rows each, 31-col overlap for window continuity) so they cost a short
    DVE pass instead of a full-width one; the resulting h segments are
    re-laid to [31, 4065] with 4 SBUF->SBUF DMAs for the final matmul group.
  * DMA queues: som/w + small stuff on the SP HWDGE ring, outputs on the ACT
    HWDGE ring; Pool does no DMA (it computes the adds for half the blocks).
"""
import numpy as np
import ml_dtypes

import concourse.bass as bass
import concourse.mybir as mybir
import concourse.tile as tile
from concourse import bacc
from concourse.bass_utils import run_bass_kernel_spmd

K = 32
HH = 4096
OUT = HH - K + 1  # 4065
N_CORES = 8
OUT_ROWS = 512
DIST_ROWS = OUT_ROWS + K - 1  # 543
STARTS = [round(c * (OUT - OUT_ROWS) / (N_CORES - 1)) for c in range(N_CORES)]

N_BLK = 4   # main 128-row blocks
N_OB = 4    # output row-blocks of 128

# halo fold geometry: 31 halo rows x 4096 cols -> [124, 1055]
# partition p = 31*seg + q holds slab row 512+q, cols COLS0[seg]..+1055
EXT_COLS0 = [0, 1024, 2048, 3041]
EXT_P = 124
EXT_W = 1055
EXT_HW = 1024  # valid h outputs per segment

F32 = mybir.dt.float32
F16 = mybir.dt.float16

# column chunks for the vertical matmul (PSUM free-dim limit 512 for f32 out)
JCHUNKS = [(j, min(512, OUT - j)) for j in range(0, OUT, 512)]

_PROGRAM_CACHE = {}


def _band_w1():
    k = np.arange(128)[:, None]
    m = np.arange(128)[None, :]
    return ((m <= k) & (k <= m + K - 1)).astype(np.float16)


def _band_w2():
    kk = np.arange(K - 1)[:, None]
    m = np.arange(128)[None, :]
    return (m >= kk + 128 - (K - 1)).astype(np.float16)


def build_program(
    repeat=1,
    pool_adds=(2,),
    interleave="skewed2",
    sq_eng="act",
    drain_pat=("aaaaaaaa", "aaaaaaaa", "adadadad", "adadadad"),
    ext_pos=2,
    out_split=2,
    bufs=(4, 4, 2, 4, 2),
):
    """pool_adds: which blocks' (som - kern) adds run on the Pool engine
    (block indices 0-3 and/or "ext"); the rest run on the DVE.
    sq_eng: "act" (ScalarE Square) or "dve" (tensor_tensor mult by itself).
    drain_pat: per mm-group 8 chars of a(ct)/d(ve)/p(ool) per PSUM chunk.
    ext_pos: how many main blocks are emitted before the halo-fold block.
    out_split: output DMAs per mm group (1 or 2), on the ACT ring."""
    nc = bacc.Bacc("TRN2", target_bir_lowering=False, debug=False)
    som = nc.dram_tensor("som", [DIST_ROWS, HH], F16, kind="ExternalInput").ap()
    wvar = nc.dram_tensor("wvar", [DIST_ROWS, HH], F16, kind="ExternalInput").ap()
    nkern = nc.dram_tensor("nkern", [128, K], F16, kind="ExternalInput").ap()
    nkern_e = nc.dram_tensor("nkern_e", [EXT_P, EXT_W], F16, kind="ExternalInput").ap()
    w1d = nc.dram_tensor("w1", [128, 128], F16, kind="ExternalInput").ap()
    w2d = nc.dram_tensor("w2", [K - 1, 128], F16, kind="ExternalInput").ap()
    out = nc.dram_tensor("out", [OUT_ROWS, OUT], F16, kind="ExternalOutput").ap()

    with tile.TileContext(nc) as tc:
        with (
            tc.tile_pool(name="const", bufs=1) as constp,
            tc.tile_pool(name="som", bufs=bufs[0]) as somp,
            tc.tile_pool(name="w", bufs=bufs[1]) as wp,
            tc.tile_pool(name="d", bufs=bufs[2]) as dp,
            tc.tile_pool(name="h0", bufs=2) as h0p,
            tc.tile_pool(name="h", bufs=bufs[3]) as hp,
            tc.tile_pool(name="ext", bufs=2) as extp,
            tc.tile_pool(name="hext", bufs=2) as hextp,
            tc.tile_pool(name="outp", bufs=bufs[4]) as outp,
            tc.tile_pool(name="psum", bufs=8, space="PSUM") as psump,
        ):
            nkern_sb = constp.tile([128, K], F16)
            nc.scalar.dma_start(nkern_sb[:], nkern[:, :])
            # broadcast view [128, 128, 32]: repeats the 32-col pattern along
            # the free dim with stride 0 so the add needs no 1 MB tiled const
            nkern_bc = nkern_sb[:].unsqueeze(1).broadcast_to([128, HH // K, K])
            nkern_e_sb = constp.tile([EXT_P, EXT_W], F16)
            nc.scalar.dma_start(nkern_e_sb[:], nkern_e[:, :])
            w1_sb = constp.tile([128, 128], F16)
            nc.scalar.dma_start(w1_sb[:], w1d[:, :])
            w2_sb = constp.tile([K - 1, 128], F16)
            nc.scalar.dma_start(w2_sb[:], w2d[:, :])

            def eng(tag):
                return nc.gpsimd if tag in pool_adds else nc.vector

            def emit_sq(t):
                if sq_eng == "act":
                    nc.scalar.activation(
                        t, t, mybir.ActivationFunctionType.Square
                    )
                else:
                    nc.vector.tensor_mul(t, t, t)

            drain_engs = {
                "a": nc.scalar,
                "d": nc.vector,
                "p": nc.gpsimd,
            }

            for _ in range(repeat):
                h_blocks = {}

                def ext_src(dram):
                    # overlapping 3-segment view [3, 31, 1055] of rows
                    # 512..542 at col starts 0/1024/2048 in ONE DMA
                    return bass.AP(
                        dram.tensor, 512 * HH, [[1024, 3], [HH, 31], [1, EXT_W]]
                    )

                def emit_ext():
                    som_t = extp.tile([EXT_P, EXT_W], F16)
                    w_t = extp.tile([EXT_P, EXT_W], F16)
                    c3 = EXT_COLS0[3]
                    with tc.high_priority():
                        nc.scalar.dma_start(som_t[0:93, :], ext_src(som))
                        nc.scalar.dma_start(
                            som_t[93:124, :], som[512:543, c3 : c3 + EXT_W]
                        )
                        nc.scalar.dma_start(w_t[0:93, :], ext_src(wvar))
                        nc.scalar.dma_start(
                            w_t[93:124, :], wvar[512:543, c3 : c3 + EXT_W]
                        )
                    eng("ext").tensor_add(som_t[:], som_t[:], nkern_e_sb[:])
                    emit_sq(som_t[:])
                    d_t = extp.tile([EXT_P, EXT_W], F16)
                    nc.vector.tensor_mul(d_t[:], som_t[:], w_t[:])
                    hseg = extp.tile([EXT_P, EXT_HW], F16)
                    h0 = h0p.tile([EXT_P, 1], F32)
                    nc.vector.tensor_reduce(
                        h0[:], d_t[:, 0:K], mybir.AxisListType.X, mybir.AluOpType.add
                    )
                    nc.vector.tensor_copy(hseg[:, 0:1], h0[:])
                    nc.vector.tensor_tensor_scan(
                        hseg[:, 1:EXT_HW],
                        d_t[:, K : K + EXT_HW - 1],
                        d_t[:, 0 : EXT_HW - 1],
                        initial=h0[:],
                        op0=mybir.AluOpType.add,
                        op1=mybir.AluOpType.subtract,
                    )
                    hext = hextp.tile([31, OUT], F16)
                    for s in range(3):
                        nc.scalar.dma_start(
                            hext[:, 1024 * s : 1024 * (s + 1)],
                            hseg[31 * s : 31 * s + 31, :],
                        )
                    nc.scalar.dma_start(
                        hext[:, 3072:OUT], hseg[93:124, 31:EXT_HW]
                    )
                    return hext

                blk_state = {}

                def stage_in(b):
                    # DMA in + diff = som + (-kern) + sq (ScalarE): emitted a
                    # block ahead of stage_scan so the square's latency hides
                    # behind the previous block's DVE work
                    rows = slice(128 * b, 128 * (b + 1))
                    som_t = somp.tile([128, HH], F16)
                    nc.sync.dma_start(som_t[:], som[rows, :])
                    w_t = wp.tile([128, HH], F16)
                    nc.sync.dma_start(w_t[:], wvar[rows, :])
                    som3d = som_t[:].rearrange("p (a b) -> p a b", b=K)
                    eng(b).tensor_add(som3d, som3d, nkern_bc)
                    emit_sq(som_t[:])
                    blk_state[b] = (som_t, w_t)

                def stage_scan(b, split=False):
                    som_t, w_t = blk_state.pop(b)
                    # d = sq * w (fp16 2x mode)
                    d_t = dp.tile([128, HH], F16)
                    nc.vector.tensor_mul(d_t[:], som_t[:], w_t[:])
                    # sliding 32-wide window sum in ONE scan pass:
                    #   h[0] = sum(d[0:32]);  h[j] = h[j-1] + d[j+31] - d[j-1]
                    # (fp16 d errors cancel exactly when an element leaves the
                    # window; only fp32 state rounding accumulates)
                    h_t = hp.tile([128, OUT], F16)
                    h0 = h0p.tile([128, 1], F32)
                    nc.vector.tensor_reduce(
                        h0[:], d_t[:, 0:K], mybir.AxisListType.X, mybir.AluOpType.add
                    )
                    nc.vector.tensor_copy(h_t[:, 0:1], h0[:])
                    if not split:
                        nc.vector.tensor_tensor_scan(
                            h_t[:, 1:OUT],
                            d_t[:, K:HH],
                            d_t[:, 0 : OUT - 1],
                            initial=h0[:],
                            op0=mybir.AluOpType.add,
                            op1=mybir.AluOpType.subtract,
                        )
                    else:
                        # state-chained halves: the first half unblocks the
                        # last mm group's left chunks while the second runs
                        nc.vector.tensor_tensor_scan(
                            h_t[:, 1:2048],
                            d_t[:, K : K + 2047],
                            d_t[:, 0:2047],
                            initial=h0[:],
                            op0=mybir.AluOpType.add,
                            op1=mybir.AluOpType.subtract,
                        )
                        nc.vector.tensor_tensor_scan(
                            h_t[:, 2048:OUT],
                            d_t[:, K + 2047 : HH],
                            d_t[:, 2047 : OUT - 1],
                            initial=h_t[:, 2047:2048],
                            op0=mybir.AluOpType.add,
                            op1=mybir.AluOpType.subtract,
                        )
                    h_blocks[b] = h_t

                def emit_out_half(ib, out_t, ci):
                    # issue the out DMA for a column half as soon as its 4
                    # drains are emitted (out_split=2), or everything at the
                    # end (out_split=1)
                    orows = slice(ib * 128, (ib + 1) * 128)
                    if out_split == 1:
                        if ci == len(JCHUNKS) - 1:
                            nc.scalar.dma_start(out[orows, :], out_t[:])
                    else:
                        if ci == 3:
                            nc.scalar.dma_start(
                                out[orows, 0:2048], out_t[:, 0:2048]
                            )
                        elif ci == len(JCHUNKS) - 1:
                            nc.scalar.dma_start(
                                out[orows, 2048:OUT], out_t[:, 2048:OUT]
                            )

                def emit_mm_group(ib, hext, w2_first=False):
                    out_t = outp.tile([128, OUT], F16)
                    h2 = hext if ib == N_OB - 1 else h_blocks[ib + 1]
                    pat = drain_pat[ib]

                    def drain(ci, j0, jw, ps):
                        de = drain_engs[pat[ci]]
                        if de is nc.scalar:
                            de.copy(out_t[:, j0 : j0 + jw], ps[:])
                        else:
                            de.tensor_copy(out_t[:, j0 : j0 + jw], ps[:])

                    if w2_first:
                        # w2 operand (hext) is ready long before the last
                        # scan: issue those 8 matmuls first so PE works
                        # while the DVE finishes h3
                        psums = []
                        for j0, jw in JCHUNKS:
                            ps = psump.tile([128, jw], F32)
                            nc.tensor.matmul(
                                ps[:], w2_sb[:], h2[: K - 1, j0 : j0 + jw],
                                start=True, stop=False,
                            )
                            psums.append(ps)
                        for ci, ((j0, jw), ps) in enumerate(zip(JCHUNKS, psums)):
                            nc.tensor.matmul(
                                ps[:], w1_sb[:], h_blocks[ib][:, j0 : j0 + jw],
                                start=False, stop=True,
                            )
                            drain(ci, j0, jw, ps)
                            emit_out_half(ib, out_t, ci)
                    else:
                        psums = []
                        for j0, jw in JCHUNKS:
                            ps = psump.tile([128, jw], F32)
                            nc.tensor.matmul(
                                ps[:], w1_sb[:], h_blocks[ib][:, j0 : j0 + jw],
                                start=True, stop=False,
                            )
                            psums.append(ps)
                        for ci, ((j0, jw), ps) in enumerate(zip(JCHUNKS, psums)):
                            nc.tensor.matmul(
                                ps[:], w2_sb[:], h2[: K - 1, j0 : j0 + jw],
                                start=False, stop=True,
                            )
                            drain(ci, j0, jw, ps)
                            emit_out_half(ib, out_t, ci)

                hext = None
                if interleave == "rot":
                    # block order 1,2,3,0: the last scan (block 0) feeds only
                    # the last mm group, whose w2 operand (h1) is ready first,
                    # so the tail is a single half-group
                    hext = emit_ext()
                    stage_in(1)
                    stage_in(2)
                    stage_scan(1, split=True)
                    stage_in(3)
                    stage_scan(2, split=True)
                    emit_mm_group(1, hext)
                    stage_in(0)
                    stage_scan(3, split=True)
                    emit_mm_group(2, hext)
                    emit_mm_group(3, hext)
                    stage_scan(0, split=True)
                    emit_mm_group(0, hext, w2_first=True)
                elif interleave == "skewed2":
                    # ext; in0; in1; scan0; in2; scan1; in3; mm0; scan2;
                    # mm1; scan3; mm2; mm3 — all scans split in halves so
                    # each mm group starts on the left chunks early
                    hext = emit_ext()
                    stage_in(0)
                    stage_in(1)
                    stage_scan(0, split=True)
                    stage_in(2)
                    stage_scan(1, split=True)
                    stage_in(3)
                    emit_mm_group(0, hext)
                    stage_scan(2, split=True)
                    emit_mm_group(1, hext)
                    stage_scan(3, split=True)
                    emit_mm_group(2, hext)
                    emit_mm_group(3, hext, w2_first=True)
                elif interleave == "skewed":
                    # ext chain first (small, unblocks mm3's w2 side), then
                    # stage_in(b+1) ahead of stage_scan(b); mm groups last
                    hext = emit_ext()
                    stage_in(0)
                    for b in range(N_BLK):
                        if b + 1 < N_BLK:
                            stage_in(b + 1)
                        stage_scan(b, split=(b == N_BLK - 1))
                    for ib in range(N_OB):
                        emit_mm_group(ib, hext, w2_first=(ib == N_OB - 1))
                elif interleave == "blocks_first":
                    # all block chains first, then all mm groups: avoids
                    # head-of-line blocking of late squares behind drains
                    for b in range(N_BLK):
                        stage_in(b); stage_scan(b)
                        if b + 1 == ext_pos:
                            hext = emit_ext()
                    if hext is None:
                        hext = emit_ext()
                    for ib in range(N_OB):
                        emit_mm_group(ib, hext)
                else:
                    hext = emit_ext()
                    for b in range(N_BLK):
                        stage_in(b); stage_scan(b)
                    for ib in range(N_OB):
                        emit_mm_group(ib, hext)

    nc.compile()
    return nc


def get_program(**kw):
    key = tuple(sorted(kw.items()))
    if key not in _PROGRAM_CACHE:
        _PROGRAM_CACHE[key] = build_program(**kw)
    return _PROGRAM_CACHE[key]


def make_in_maps(input_image, som_matrix, som_running_variances):
    img = np.asarray(input_image, dtype=np.float32)
    som = np.asarray(som_matrix, dtype=np.float32)
    var = np.asarray(som_running_variances, dtype=np.float32)
    w_full = 1.0 / (var + 1e-8)
    w1 = np.ascontiguousarray(_band_w1())
    w2 = np.ascontiguousarray(_band_w2())
    in_maps = []
    q_idx = np.arange(EXT_P) % 31
    s_idx = np.arange(EXT_P) // 31
    j_idx = np.arange(EXT_W)
    for c in range(N_CORES):
        s = STARTS[c]
        # slab-local row i is global row s+i -> kern row img[(s+i) % K]
        # only [128, 32] is shipped; the device broadcasts along columns
        negkern = np.ascontiguousarray(
            (-np.tile(np.roll(img, -(s % K), axis=0), (128 // K, 1))).astype(
                np.float16
            )
        )
        # ext tile rows: slab row 512+q (512%32==0), cols COLS0[seg]+j
        nkern_e = (
            -img[
                (s + q_idx[:, None]) % K,
                (np.asarray(EXT_COLS0)[s_idx][:, None] + j_idx[None, :]) % K,
            ]
        ).astype(np.float16)
        in_maps.append(
            {
                "som": np.ascontiguousarray(som[s : s + DIST_ROWS].astype(np.float16)),
                "wvar": np.ascontiguousarray(
                    w_full[s : s + DIST_ROWS].astype(np.float16)
                ),
                "nkern": negkern,
                "nkern_e": np.ascontiguousarray(nkern_e),
                "w1": w1,
                "w2": w2,
            }
        )
    return in_maps


def assemble(results):
    out_full = np.empty((OUT, OUT), np.float32)
    for c in range(N_CORES):
        lo = STARTS[c]
        hi = STARTS[c + 1] if c < N_CORES - 1 else OUT
        out_full[lo:hi] = results[c]["out"][: hi - lo].astype(np.float32)
    return out_full


def kernel(input_image, som_matrix, som_running_variances):
    nc = get_program()
    in_maps = make_in_maps(input_image, som_matrix, som_running_variances)
    res = run_bass_kernel_spmd(nc, in_maps, core_ids=list(range(N_CORES)))
    return assemble(res.results)
